# revision 1
# baseline (speedup 1.0000x reference)
"""Trainium2 kernel for the FEM kinematic (strain) layer.

Reference computation:
    disp = inputs[:, elem_nodes]                      # [B, E, 8, 2]
    dd   = einsum('egkl,bekn->begnl', shpdx, disp)    # [B, E, 9, 2, 2]
    out  = stack([dd[...,0,0], dd[...,1,1],
                  0.5*(dd[...,0,1] + dd[...,1,0])])   # [B, E*9, 3]

Sharding: elements split across 8 NeuronCores.  The host resolves the
element->node indirection (index marshalling) and ships each core an
element-major displacement block; the device streams shpdx + disp and
computes the strain products with DVE, using the identity
    S1*u + S0*v = (S0+S1)*(u+v) - S0*u - S1*v
so only 3 elementwise products are needed per (element, gauss point).
"""

import sys
import numpy as np

sys.path.insert(0, "/opt/trn_rl_repo")

import concourse.bass as bass
import concourse.bacc as bacc
import concourse.mybir as mybir
import concourse.tile as tile
from concourse.bass_utils import run_bass_kernel_spmd

B = 4
N_NODES = 1_000_000
N_ELEM = 500_000
N_GP = 9
N_EN = 8
N_CORES = 8

E_CORE = N_ELEM // N_CORES            # 62500 elements per core
P = 128                               # SBUF partitions
C = 16                                # elements per partition per chunk
CHUNK = P * C                         # 2048 elements per chunk
N_CHUNKS = -(-E_CORE // CHUNK)        # 31
E_PAD = N_CHUNKS * CHUNK              # 63488 (988 pad elements)

_compiled = None


def _build_program():
    nc = bacc.Bacc("TRN2", target_bir_lowering=False, debug=False)
    f32 = mybir.dt.float32

    # [E_PAD, 144] f32: per element (g, k, l) row-major
    s_d = nc.dram_tensor("shp", [E_PAD, 144], f32, kind="ExternalInput").ap()
    # [E_PAD, 64] f32: per element (k, b, n) row-major
    d_d = nc.dram_tensor("disp", [E_PAD, 64], f32, kind="ExternalInput").ap()
    # [B, E_PAD*9, 3] f32
    o_d = nc.dram_tensor("out", [B, E_PAD * 9, 3], f32, kind="ExternalOutput").ap()

    s_v = s_d.rearrange("(n p c) f -> n p (c f)", p=P, c=C)
    d_v = d_d.rearrange("(n p c) f -> n p (c f)", p=P, c=C)
    # out view per (b, chunk): [P, C*27]
    o_v = o_d.rearrange("b (n p x) three -> b n p (x three)", p=P, x=C * 9)

    with tile.TileContext(nc) as tc:
        with (
            tc.tile_pool(name="io", bufs=4) as io_pool,
            tc.tile_pool(name="tmp", bufs=3) as tmp_pool,
        ):
            for i in range(N_CHUNKS):
                S = io_pool.tile([P, C * 144], f32, tag="S")
                D = io_pool.tile([P, C * 64], f32, tag="D")
                nc.sync.dma_start(out=S[:], in_=s_v[i])
                nc.sync.dma_start(out=D[:], in_=d_v[i])

                Sr = S[:].rearrange("p (c g k l) -> p c g k l", c=C, g=9, k=8, l=2)
                Dr = D[:].rearrange("p (c k b n) -> p c k b n", c=C, k=8, b=B, n=2)

                # A = S0 + S1, contiguous [p, (c g k)]
                A = tmp_pool.tile([P, C * 72], f32, tag="A")
                Av = A[:].rearrange("p (c g k) -> p c g k", c=C, g=9)
                nc.vector.tensor_tensor(
                    out=Av, in0=Sr[:, :, :, :, 0], in1=Sr[:, :, :, :, 1],
                    op=mybir.AluOpType.add,
                )

                O = io_pool.tile([P, B * C * 27], f32, tag="O")
                Ov = O[:].rearrange("p (b c g t) -> p b c g t", b=B, c=C, g=9, t=3)

                for b in range(B):
                    u = Dr[:, :, :, b, 0]          # [p, C, 8]
                    v = Dr[:, :, :, b, 1]

                    W = tmp_pool.tile([P, C * 8], f32, tag="W")
                    Wv = W[:].rearrange("p (c k) -> p c k", c=C)
                    nc.gpsimd.tensor_tensor(
                        out=Wv, in0=u, in1=v, op=mybir.AluOpType.add
                    )

                    # broadcast displacement over g: [p, C, 1, 8] -> [p, C, 9, 8]
                    ub = u[:, :, None, :].to_broadcast([P, C, 9, 8])
                    vb = v[:, :, None, :].to_broadcast([P, C, 9, 8])
                    wb = Wv[:, :, None, :].to_broadcast([P, C, 9, 8])

                    T0 = tmp_pool.tile([P, C * 72], f32, tag="T")
                    T1 = tmp_pool.tile([P, C * 72], f32, tag="T")
                    T2 = tmp_pool.tile([P, C * 72], f32, tag="T")
                    T0v = T0[:].rearrange("p (c g k) -> p c g k", c=C, g=9)
                    T1v = T1[:].rearrange("p (c g k) -> p c g k", c=C, g=9)
                    T2v = T2[:].rearrange("p (c g k) -> p c g k", c=C, g=9)

                    nc.vector.tensor_tensor(
                        out=T0v, in0=Sr[:, :, :, :, 0], in1=ub,
                        op=mybir.AluOpType.mult,
                    )
                    nc.vector.tensor_tensor(
                        out=T1v, in0=Sr[:, :, :, :, 1], in1=vb,
                        op=mybir.AluOpType.mult,
                    )
                    nc.vector.tensor_tensor(
                        out=T2v, in0=Av, in1=wb, op=mybir.AluOpType.mult,
                    )

                    # xx / yy land directly in the (strided) output staging
                    nc.vector.reduce_sum(
                        out=Ov[:, b, :, :, 0], in_=T0v, axis=mybir.AxisListType.X
                    )
                    nc.vector.reduce_sum(
                        out=Ov[:, b, :, :, 1], in_=T1v, axis=mybir.AxisListType.X
                    )

                    R = tmp_pool.tile([P, C * 9], f32, tag="R")
                    Rv = R[:].rearrange("p (c g) -> p c g", c=C)
                    nc.vector.reduce_sum(out=Rv, in_=T2v, axis=mybir.AxisListType.X)
                    nc.gpsimd.tensor_tensor(
                        out=Rv, in0=Rv, in1=Ov[:, b, :, :, 0],
                        op=mybir.AluOpType.subtract,
                    )
                    nc.gpsimd.tensor_tensor(
                        out=Rv, in0=Rv, in1=Ov[:, b, :, :, 1],
                        op=mybir.AluOpType.subtract,
                    )
                    nc.scalar.activation(
                        out=Ov[:, b, :, :, 2], in_=Rv,
                        func=mybir.ActivationFunctionType.Copy, scale=0.5,
                    )

                for b in range(B):
                    nc.sync.dma_start(
                        out=o_v[b, i],
                        in_=O[:, b * C * 27:(b + 1) * C * 27],
                    )

    nc.compile()
    return nc


def _get_program():
    global _compiled
    if _compiled is None:
        _compiled = _build_program()
    return _compiled


def kernel(inputs, shpdx, elem_nodes, _want_trace=False):
    nc = _get_program()

    # Host-side index marshalling: resolve element->node indirection and
    # build per-core element-major blocks.
    in_maps = []
    for c in range(N_CORES):
        sl = slice(c * E_CORE, (c + 1) * E_CORE)
        en = elem_nodes[sl]                                   # [E, 8]
        disp = inputs[:, en]                                  # [B, E, 8, 2]
        dispc = np.ascontiguousarray(disp.transpose(1, 2, 0, 3))  # [E, 8, B, 2]
        dispc = dispc.reshape(E_CORE, 64)
        dpad = np.zeros((E_PAD, 64), np.float32)
        dpad[:E_CORE] = dispc
        spad = np.zeros((E_PAD, 144), np.float32)
        spad[:E_CORE] = shpdx[sl].reshape(E_CORE, 144)
        in_maps.append({"shp": spad, "disp": dpad})

    core_ids = list(range(N_CORES))
    res = run_bass_kernel_spmd(nc, in_maps, core_ids, trace=_want_trace)

    outs = []
    for c in range(N_CORES):
        o = res.results[c]["out"]                             # [B, E_PAD*9, 3]
        outs.append(o[:, :E_CORE * 9, :])
    full = np.concatenate(outs, axis=1)                       # [B, N_ELEM*9, 3]
    if _want_trace:
        return full, res
    return full



# revision 2
# speedup vs baseline: 6.9711x; 6.9711x over previous
"""Trainium2 kernel for the FEM kinematic (strain) layer.

Reference computation:
    disp = inputs[:, elem_nodes]                      # [B, E, 8, 2]
    dd   = einsum('egkl,bekn->begnl', shpdx, disp)    # [B, E, 9, 2, 2]
    out  = stack([dd[...,0,0], dd[...,1,1],
                  0.5*(dd[...,0,1] + dd[...,1,0])])   # [B, E*9, 3]

Strategy: elements are split across 8 NeuronCores.  The host resolves the
element->node indirection and ships fp16 per-element displacement/shape-
derivative blocks in a partition-major layout.  On the device the per-element
contraction over the 8 nodes runs on the TensorEngine: 16 elements at a time
are packed as 8x8 blocks on the block diagonal of a 128x128 stationary
operand (built on the VectorEngine as broadcast*mask in one fp16 2x op), and
four 32x32 tile_position matmuls per round compute
    dd[(el,b,n), (g,l)] = sum_k disp[el][k,(b,n)] * shpdx[el][(g,l),k]
into PSUM.  Full PSUM banks (28 rounds = 448 elements) are evicted once as
fp16 by the ScalarEngine and DMA'd out.  The host unpacks dd and combines the
strain components.
"""

import sys

import numpy as np

sys.path.insert(0, "/opt/trn_rl_repo")

import concourse.bacc as bacc
import concourse.mybir as mybir
import concourse.tile as tile
from concourse.bass_utils import run_bass_kernel_spmd

B = 4
N_NODES = 1_000_000
N_ELEM = 500_000
N_GP = 9
N_EN = 8
N_CORES = 8

P = 128
NR = 28                    # rounds (16 elements each) per PSUM bank
EPB = 16 * NR              # elements per bank = 448
NBANK = 140                # banks per core
E_CORE = N_ELEM // N_CORES      # 62500
E_PAD = NBANK * EPB             # 62720
G = 10                     # banks per input DMA group
EG = 4                     # banks per output DMA group
NBG = NBANK // G           # 14
NEVG = NBANK // EG         # 35

_compiled = None


def _build_program():
    nc = bacc.Bacc("TRN2", target_bir_lowering=False, debug=False)
    f16 = mybir.dt.float16
    f32 = mybir.dt.float32

    # disp, dense:  [bg, p=(i,el2,k), gi, r, (b,n)]
    d_d = nc.dram_tensor("d_in", [NBG, P, G, NR, 8], f16, kind="ExternalInput").ap()
    # shpdx, rhs-ready: [bg, p=(i,el2,k), gi, r, (g,l)]
    s_d = nc.dram_tensor("s_in", [NBG, P, G, NR, 18], f16, kind="ExternalInput").ap()
    # block-diag mask: [p, (el2', bn)]
    m_d = nc.dram_tensor("mask", [P, 32], f16, kind="ExternalInput").ap()
    # dd out: [ev, p=(i, el2', b, n), eslot, r, (g,l)]
    o_d = nc.dram_tensor("out", [NEVG, P, EG, NR, 18], f16, kind="ExternalOutput").ap()

    with tile.TileContext(nc) as tc:
        with (
            tc.tile_pool(name="const", bufs=1) as const_pool,
            tc.tile_pool(name="io", bufs=3) as io_pool,
            tc.tile_pool(name="w", bufs=4) as w_pool,
            tc.tile_pool(name="ps", bufs=6, space="PSUM") as ps_pool,
            tc.tile_pool(name="ev", bufs=3) as ev_pool,
        ):
            Mt = const_pool.tile([P, 32], f16, tag="M")
            nc.sync.dma_start(out=Mt[:], in_=m_d)
            Mv = Mt[:].rearrange("p (e b) -> p e b", e=4)

            ev = None
            for bg in range(NBG):
                D = io_pool.tile([P, G * NR * 8], f16, tag="D")
                S = io_pool.tile([P, G * NR * 18], f16, tag="S")
                nc.sync.dma_start(out=D[:], in_=d_d[bg].rearrange("p g r b -> p (g r b)"))
                nc.sync.dma_start(out=S[:], in_=s_d[bg].rearrange("p g r b -> p (g r b)"))

                for gi in range(G):
                    bk = bg * G + gi
                    if bk % EG == 0:
                        ev = ev_pool.tile([P, EG * NR * 18], f16, tag="ev")

                    # W[p, (r, el2', bn)] = D[p, (gi, r, bn)] * M[p, (el2', bn)]
                    W = w_pool.tile([P, NR * 32], f16, tag="W")
                    Wv = W[:].rearrange("p (r e b) -> p r e b", r=NR, e=4)
                    Dv = D[:].rearrange("p (g r b) -> p g r b", g=G, r=NR)
                    Db = Dv[:, gi, :, None, :].to_broadcast([P, NR, 4, 8])
                    Mb = Mv[:, None, :, :].to_broadcast([P, NR, 4, 8])
                    nc.vector.tensor_tensor(out=Wv, in0=Db, in1=Mb,
                                            op=mybir.AluOpType.mult)

                    ps = ps_pool.tile([P, 512], f32, tag="ps")
                    for r in range(NR):
                        for i in range(4):
                            pr = slice(32 * i, 32 * i + 32)
                            nc.tensor.matmul(
                                out=ps[pr, r * 18:(r + 1) * 18],
                                lhsT=W[pr, r * 32:(r + 1) * 32],
                                rhs=S[pr, (gi * NR + r) * 18:(gi * NR + r + 1) * 18],
                                start=True, stop=True,
                                tile_position=(32 * i, 32 * i),
                            )

                    eslot = bk % EG
                    nc.scalar.copy(
                        out=ev[:, eslot * NR * 18:(eslot + 1) * NR * 18],
                        in_=ps[:, :NR * 18])

                    if eslot == EG - 1:
                        nc.sync.dma_start(
                            out=o_d[bk // EG].rearrange("p e r g -> p (e r g)"),
                            in_=ev[:])

    nc.compile()
    return nc


def _get_program():
    global _compiled
    if _compiled is None:
        _compiled = _build_program()
    return _compiled


def _make_mask() -> np.ndarray:
    m = np.zeros((P, 32), np.float16)
    for p in range(P):
        el2 = (p % 32) // 8
        m[p, el2 * 8:(el2 + 1) * 8] = 1.0
    return m


def _marshal_core(inputs_f16: np.ndarray, shpdx: np.ndarray,
                  elem_nodes: np.ndarray, c: int):
    """Build the d_in / s_in arrays for core c."""
    sl = slice(c * E_CORE, (c + 1) * E_CORE)
    en = elem_nodes[sl]                                   # [E, 8]
    disp = inputs_f16[:, en]                              # [B, E, 8, 2] f16
    # -> [E, k, (b, n)]
    dispc = np.ascontiguousarray(disp.transpose(1, 2, 0, 3)).reshape(E_CORE, 8, 8)
    dpad = np.zeros((E_PAD, 8, 8), np.float16)
    dpad[:E_CORE] = dispc
    # e = (((bg*G + gi)*NR + r)*4 + i)*4 + el2
    dh = dpad.reshape(NBG, G, NR, 4, 4, 8, 8)             # bg gi r i el2 k bn
    dh = np.ascontiguousarray(dh.transpose(0, 3, 4, 5, 1, 2, 6))  # bg i el2 k gi r bn
    d_in = dh.reshape(NBG, P, G, NR, 8)

    spad = np.zeros((E_PAD, N_GP, 8, 2), np.float16)
    spad[:E_CORE] = shpdx[sl].astype(np.float16)
    sh = spad.reshape(NBG, G, NR, 4, 4, N_GP, 8, 2)       # bg gi r i el2 g k l
    sh = np.ascontiguousarray(sh.transpose(0, 3, 4, 6, 1, 2, 5, 7))  # bg i el2 k gi r g l
    s_in = sh.reshape(NBG, P, G, NR, 18)
    return d_in, s_in


def _decode_core(o: np.ndarray) -> np.ndarray:
    """o: [NEVG, P, EG, NR, 18] fp16 -> strains [B, E_CORE*9, 3] f32."""
    # p = 32i + 8el2 + 2b + n ; e = (((ev*EG + eslot)*NR + r)*4 + i)*4 + el2
    oh = o.reshape(NEVG, 4, 4, B, 2, EG, NR, N_GP, 2)     # ev i el2 b n es r g l
    oh = oh.transpose(3, 0, 5, 6, 1, 2, 7, 4, 8)          # b ev es r i el2 g n l
    dd = np.ascontiguousarray(oh).reshape(B, E_PAD, N_GP, 2, 2).astype(np.float32)
    dd = dd[:, :E_CORE]
    e_xx = dd[..., 0, 0]
    e_yy = dd[..., 1, 1]
    e_xy = 0.5 * (dd[..., 0, 1] + dd[..., 1, 0])
    e = np.stack([e_xx, e_yy, e_xy], axis=-1)             # [B, E, 9, 3]
    return e.reshape(B, E_CORE * N_GP, 3)


def kernel(inputs, shpdx, elem_nodes, _want_trace=False):
    nc = _get_program()

    inputs_f16 = inputs.astype(np.float16)
    mask = _make_mask()
    in_maps = []
    for c in range(N_CORES):
        d_in, s_in = _marshal_core(inputs_f16, shpdx, elem_nodes, c)
        in_maps.append({"d_in": d_in, "s_in": s_in, "mask": mask})

    core_ids = list(range(N_CORES))
    res = run_bass_kernel_spmd(nc, in_maps, core_ids, trace=_want_trace)

    outs = []
    for c in range(N_CORES):
        outs.append(_decode_core(np.asarray(res.results[c]["out"])))
    full = np.concatenate(outs, axis=1)                   # [B, N_ELEM*9, 3]
    if _want_trace:
        return full, res
    return full


# revision 4
# speedup vs baseline: 7.4428x; 1.0677x over previous
"""Trainium2 kernel for the FEM kinematic (strain) layer.

Reference computation:
    disp = inputs[:, elem_nodes]                      # [B, E, 8, 2]
    dd   = einsum('egkl,bekn->begnl', shpdx, disp)    # [B, E, 9, 2, 2]
    out  = stack([dd[...,0,0], dd[...,1,1],
                  0.5*(dd[...,0,1] + dd[...,1,0])])   # [B, E*9, 3]

Strategy: elements split across 8 NeuronCores.  The host resolves the
element->node indirection and ships fp16 per-element blocks in a
partition-major layout.  On the device the per-element contraction over the
8 nodes runs on the TensorEngine: 16 elements per round are packed as 8x8
blocks on the diagonal of the stationary operand (built on the VectorEngine
as broadcast*mask in one fp16 2x op), and tile_position matmuls compute
    dd[(el,b,n), (g,l)] = sum_k disp[el][k,(b,n)] * shpdx[el][(g,l),k]
into PSUM.  Banks alternate between 4x(32x32) and 2x(64x64) subarray
splits so the VectorE (mask build, cost ~ W columns) and TensorE (cost ~
output columns) loads average out below the DMA roofline.  Full PSUM banks
(28 rounds = 448 elements) are evicted once as fp16 by the ScalarEngine and
DMA'd out; the host unpacks dd and combines the strain components.
"""

import sys

import numpy as np

sys.path.insert(0, "/opt/trn_rl_repo")

import concourse.bacc as bacc
import concourse.mybir as mybir
import concourse.tile as tile
from concourse.bass_utils import run_bass_kernel_spmd

B = 4
N_NODES = 1_000_000
N_ELEM = 500_000
N_GP = 9
N_EN = 8
N_CORES = 8

P = 128
NR = 28                    # rounds (16 elements each) per PSUM bank
EPB = 16 * NR              # elements per bank = 448
NBANK = 140                # banks per core
E_CORE = N_ELEM // N_CORES      # 62500
E_PAD = NBANK * EPB             # 62720
G = 10                     # banks per input DMA group
EG = 4                     # banks per output DMA group
NBG = NBANK // G           # 14
NEVG = NBANK // EG         # 35


def _is_sub2(bk: int) -> bool:
    return bk % 2 == 0


_compiled = None


def _build_program():
    nc = bacc.Bacc("TRN2", target_bir_lowering=False, debug=False)
    f16 = mybir.dt.float16
    f32 = mybir.dt.float32

    # disp, dense:  [bg, p=(grp,el,k), gi, r, (b,n)]
    d_d = nc.dram_tensor("d_in", [NBG, P, G, NR, 8], f16, kind="ExternalInput").ap()
    # shpdx, rhs-ready: [bg, p=(grp,el,k), gi, r, (g,l)]
    s_d = nc.dram_tensor("s_in", [NBG, P, G, NR, 18], f16, kind="ExternalInput").ap()
    # block-diag masks
    m_d = nc.dram_tensor("mask", [P, 32], f16, kind="ExternalInput").ap()
    m2_d = nc.dram_tensor("mask2", [P, 64], f16, kind="ExternalInput").ap()
    # dd out: [ev, p=(grp, el', b, n), eslot, r, (g,l)]
    o_d = nc.dram_tensor("out", [NEVG, P, EG, NR, 18], f16, kind="ExternalOutput").ap()

    with tile.TileContext(nc) as tc:
        with (
            tc.tile_pool(name="const", bufs=1) as const_pool,
            tc.tile_pool(name="io", bufs=3) as io_pool,
            tc.tile_pool(name="w", bufs=4) as w_pool,
            tc.tile_pool(name="ps", bufs=6, space="PSUM") as ps_pool,
            tc.tile_pool(name="ev", bufs=3) as ev_pool,
        ):
            Mt = const_pool.tile([P, 32], f16, tag="M")
            nc.sync.dma_start(out=Mt[:], in_=m_d)
            Mv = Mt[:].rearrange("p (e b) -> p e b", e=4)
            M2t = const_pool.tile([P, 64], f16, tag="M2")
            nc.sync.dma_start(out=M2t[:], in_=m2_d)
            M2v = M2t[:].rearrange("p (e b) -> p e b", e=8)

            ev = None
            for bg in range(NBG):
                D = io_pool.tile([P, G * NR * 8], f16, tag="D")
                S = io_pool.tile([P, G * NR * 18], f16, tag="S")
                nc.sync.dma_start(out=D[:], in_=d_d[bg].rearrange("p g r b -> p (g r b)"))
                nc.sync.dma_start(out=S[:], in_=s_d[bg].rearrange("p g r b -> p (g r b)"))

                for gi in range(G):
                    bk = bg * G + gi
                    if bk % EG == 0:
                        ev = ev_pool.tile([P, EG * NR * 18], f16, tag="ev")

                    Dv = D[:].rearrange("p (g r b) -> p g r b", g=G, r=NR)
                    ps = ps_pool.tile([P, 512], f32, tag="ps")

                    if _is_sub2(bk):
                        # W[p, (r, el', bn)] = D[p, (gi, r, bn)] * M2[p, (el', bn)]
                        W = w_pool.tile([P, NR * 64], f16, tag="W2")
                        Wv = W[:].rearrange("p (r e b) -> p r e b", r=NR, e=8)
                        Db = Dv[:, gi, :, None, :].to_broadcast([P, NR, 8, 8])
                        Mb = M2v[:, None, :, :].to_broadcast([P, NR, 8, 8])
                        nc.vector.tensor_tensor(out=Wv, in0=Db, in1=Mb,
                                                op=mybir.AluOpType.mult)
                        for r in range(NR):
                            for h in range(2):
                                pr = slice(64 * h, 64 * h + 64)
                                nc.tensor.matmul(
                                    out=ps[pr, r * 18:(r + 1) * 18],
                                    lhsT=W[pr, r * 64:(r + 1) * 64],
                                    rhs=S[pr, (gi * NR + r) * 18:(gi * NR + r + 1) * 18],
                                    start=True, stop=True,
                                    tile_position=(64 * h, 64 * h),
                                )
                    else:
                        W = w_pool.tile([P, NR * 32], f16, tag="W")
                        Wv = W[:].rearrange("p (r e b) -> p r e b", r=NR, e=4)
                        Db = Dv[:, gi, :, None, :].to_broadcast([P, NR, 4, 8])
                        Mb = Mv[:, None, :, :].to_broadcast([P, NR, 4, 8])
                        nc.vector.tensor_tensor(out=Wv, in0=Db, in1=Mb,
                                                op=mybir.AluOpType.mult)
                        for r in range(NR):
                            for i in range(4):
                                pr = slice(32 * i, 32 * i + 32)
                                nc.tensor.matmul(
                                    out=ps[pr, r * 18:(r + 1) * 18],
                                    lhsT=W[pr, r * 32:(r + 1) * 32],
                                    rhs=S[pr, (gi * NR + r) * 18:(gi * NR + r + 1) * 18],
                                    start=True, stop=True,
                                    tile_position=(32 * i, 32 * i),
                                )

                    eslot = bk % EG
                    nc.scalar.copy(
                        out=ev[:, eslot * NR * 18:(eslot + 1) * NR * 18],
                        in_=ps[:, :NR * 18])

                    if eslot == EG - 1:
                        nc.sync.dma_start(
                            out=o_d[bk // EG].rearrange("p e r g -> p (e r g)"),
                            in_=ev[:])

    nc.compile()
    return nc


def _get_program():
    global _compiled
    if _compiled is None:
        _compiled = _build_program()
    return _compiled


def _make_masks():
    m = np.zeros((P, 32), np.float16)
    m2 = np.zeros((P, 64), np.float16)
    for p in range(P):
        el4 = (p % 32) // 8
        m[p, el4 * 8:(el4 + 1) * 8] = 1.0
        el8 = (p % 64) // 8
        m2[p, el8 * 8:(el8 + 1) * 8] = 1.0
    return m, m2


_SUB2_BANKS = np.array([bk for bk in range(NBANK) if _is_sub2(bk)])
_SUB4_BANKS = np.array([bk for bk in range(NBANK) if not _is_sub2(bk)])


def _marshal_core(inputs_f16: np.ndarray, shpdx: np.ndarray,
                  elem_nodes: np.ndarray, c: int):
    """Build the d_in / s_in arrays for core c."""
    sl = slice(c * E_CORE, (c + 1) * E_CORE)
    en = elem_nodes[sl]                                   # [E, 8]
    disp = inputs_f16[:, en]                              # [B, E, 8, 2] f16
    # -> [E, k, (b, n)]
    dispc = np.ascontiguousarray(disp.transpose(1, 2, 0, 3)).reshape(E_CORE, 8, 8)
    dpad = np.zeros((E_PAD, 8, 8), np.float16)
    dpad[:E_CORE] = dispc
    spad = np.zeros((E_PAD, N_GP, 8, 2), np.float16)
    spad[:E_CORE] = shpdx[sl].astype(np.float16)

    # e = bk*EPB + r*16 + grp*per + el ; partition p = grp*per*8 + el*8 + k
    d_all = np.empty((NBANK, P, NR, 8), np.float16)
    s_all = np.empty((NBANK, P, NR, 18), np.float16)
    db = dpad.reshape(NBANK, NR, 16, 8, 8)                # bk r sub k bn
    sb = spad.reshape(NBANK, NR, 16, N_GP, 8, 2)          # bk r sub g k l
    for banks, ngrp, per in ((_SUB2_BANKS, 2, 8), (_SUB4_BANKS, 4, 4)):
        dv = db[banks].reshape(len(banks), NR, ngrp, per, 8, 8)
        dv = dv.transpose(0, 2, 3, 4, 1, 5)               # bk grp el k r bn
        d_all[banks] = dv.reshape(len(banks), P, NR, 8)
        sv = sb[banks].reshape(len(banks), NR, ngrp, per, N_GP, 8, 2)
        sv = sv.transpose(0, 2, 3, 5, 1, 4, 6)            # bk grp el k r g l
        s_all[banks] = sv.reshape(len(banks), P, NR, 18)

    d_in = np.ascontiguousarray(
        d_all.reshape(NBG, G, P, NR, 8).transpose(0, 2, 1, 3, 4))
    s_in = np.ascontiguousarray(
        s_all.reshape(NBG, G, P, NR, 18).transpose(0, 2, 1, 3, 4))
    return d_in, s_in


def _decode_core(o: np.ndarray) -> np.ndarray:
    """o: [NEVG, P, EG, NR, 18] fp16 -> strains [B, E_CORE*9, 3] f32."""
    # out partition p = grp*per*8 + el'*8 + (b*2+n)
    ob = o.reshape(NEVG, P, EG, NR, N_GP, 2).transpose(0, 2, 1, 3, 4, 5)
    ob = ob.reshape(NBANK, P, NR, N_GP, 2)                # bk p r g l
    dd = np.empty((B, E_PAD, N_GP, 2, 2), np.float16)
    ddv = dd.reshape(B, NBANK, NR, 16, N_GP, 2, 2)
    for banks, ngrp, per in ((_SUB2_BANKS, 2, 8), (_SUB4_BANKS, 4, 4)):
        ov = ob[banks].reshape(len(banks), ngrp, per, B, 2, NR, N_GP, 2)
        # -> b bk r (grp el) g n l
        ov = ov.transpose(3, 0, 5, 1, 2, 6, 4, 7)
        ddv[:, banks] = ov.reshape(B, len(banks), NR, 16, N_GP, 2, 2)
    dd = dd[:, :E_CORE].astype(np.float32)
    e_xx = dd[..., 0, 0]
    e_yy = dd[..., 1, 1]
    e_xy = 0.5 * (dd[..., 0, 1] + dd[..., 1, 0])
    e = np.stack([e_xx, e_yy, e_xy], axis=-1)             # [B, E, 9, 3]
    return e.reshape(B, E_CORE * N_GP, 3)


def kernel(inputs, shpdx, elem_nodes, _want_trace=False):
    nc = _get_program()

    inputs_f16 = inputs.astype(np.float16)
    mask, mask2 = _make_masks()
    in_maps = []
    for c in range(N_CORES):
        d_in, s_in = _marshal_core(inputs_f16, shpdx, elem_nodes, c)
        in_maps.append({"d_in": d_in, "s_in": s_in, "mask": mask, "mask2": mask2})

    core_ids = list(range(N_CORES))
    res = run_bass_kernel_spmd(nc, in_maps, core_ids, trace=_want_trace)

    outs = []
    for c in range(N_CORES):
        outs.append(_decode_core(np.asarray(res.results[c]["out"])))
    full = np.concatenate(outs, axis=1)                   # [B, N_ELEM*9, 3]
    if _want_trace:
        return full, res
    return full


# revision 5
# speedup vs baseline: 7.4851x; 1.0057x over previous
"""Trainium2 kernel for the FEM kinematic (strain) layer.

Reference computation:
    disp = inputs[:, elem_nodes]                      # [B, E, 8, 2]
    dd   = einsum('egkl,bekn->begnl', shpdx, disp)    # [B, E, 9, 2, 2]
    out  = stack([dd[...,0,0], dd[...,1,1],
                  0.5*(dd[...,0,1] + dd[...,1,0])])   # [B, E*9, 3]

Strategy: elements split across 8 NeuronCores.  The host resolves the
element->node indirection and ships fp16 per-element blocks in a
partition-major layout.  On the device the per-element contraction over the
8 nodes runs on the TensorEngine: 16 elements per round are packed as 8x8
blocks on the diagonal of the stationary operand (built on the VectorEngine
as broadcast*mask in one fp16 2x op), and tile_position matmuls compute
    dd[(el,b,n), (g,l)] = sum_k disp[el][k,(b,n)] * shpdx[el][(g,l),k]
into PSUM.  Banks alternate between 4x(32x32) and 2x(64x64) subarray
splits so the VectorE (mask build, cost ~ W columns) and TensorE (cost ~
output columns) loads average out below the DMA roofline.  Full PSUM banks
(28 rounds = 448 elements) are evicted once as fp16 by the ScalarEngine and
DMA'd out; the host unpacks dd and combines the strain components.
"""

import sys

import numpy as np

sys.path.insert(0, "/opt/trn_rl_repo")

import concourse.bacc as bacc
import concourse.mybir as mybir
import concourse.tile as tile
from concourse.bass_utils import run_bass_kernel_spmd

B = 4
N_NODES = 1_000_000
N_ELEM = 500_000
N_GP = 9
N_EN = 8
N_CORES = 8

P = 128
NR = 28                    # rounds (16 elements each) per PSUM bank
EPB = 16 * NR              # elements per bank = 448
NBANK = 140                # banks per core
E_CORE = N_ELEM // N_CORES      # 62500
E_PAD = NBANK * EPB             # 62720
G = 10                     # banks per input DMA group
EG = 4                     # banks per output DMA group
NBG = NBANK // G           # 14
NEVG = NBANK // EG         # 35


def _is_sub2(bk: int) -> bool:
    return bk % 2 == 0


_compiled = None


def _build_program():
    nc = bacc.Bacc("TRN2", target_bir_lowering=False, debug=False)
    f16 = mybir.dt.float16
    f32 = mybir.dt.float32

    # disp, dense:  [bg, p=(grp,el,k), gi, r, (b,n)]
    d_d = nc.dram_tensor("d_in", [NBG, P, G, NR, 8], f16, kind="ExternalInput").ap()
    # shpdx, rhs-ready: [bg, p=(grp,el,k), gi, r, (g,l)]
    s_d = nc.dram_tensor("s_in", [NBG, P, G, NR, 18], f16, kind="ExternalInput").ap()
    # block-diag masks, packed: cols 0:32 sub4, cols 32:96 sub2
    m_d = nc.dram_tensor("mask", [P, 96], f16, kind="ExternalInput").ap()
    # dd out: [ev, p=(grp, el', b, n), eslot, r, (g,l)]
    o_d = nc.dram_tensor("out", [NEVG, P, EG, NR, 18], f16, kind="ExternalOutput").ap()

    with tile.TileContext(nc) as tc:
        with (
            tc.tile_pool(name="const", bufs=1) as const_pool,
            tc.tile_pool(name="io", bufs=3) as io_pool,
            tc.tile_pool(name="w", bufs=4) as w_pool,
            tc.tile_pool(name="ps", bufs=6, space="PSUM") as ps_pool,
            tc.tile_pool(name="ev", bufs=3) as ev_pool,
        ):
            Mt = const_pool.tile([P, 96], f16, tag="M")
            nc.sync.dma_start(out=Mt[:], in_=m_d)
            Mv = Mt[:, :32].rearrange("p (e b) -> p e b", e=4)
            M2v = Mt[:, 32:].rearrange("p (e b) -> p e b", e=8)

            ev = None
            for bg in range(NBG):
                D = io_pool.tile([P, G * NR * 8], f16, tag="D")
                S = io_pool.tile([P, G * NR * 18], f16, tag="S")
                nc.sync.dma_start(out=D[:], in_=d_d[bg].rearrange("p g r b -> p (g r b)"))
                nc.sync.dma_start(out=S[:], in_=s_d[bg].rearrange("p g r b -> p (g r b)"))

                for gi in range(G):
                    bk = bg * G + gi
                    if bk % EG == 0:
                        ev = ev_pool.tile([P, EG * NR * 18], f16, tag="ev")

                    Dv = D[:].rearrange("p (g r b) -> p g r b", g=G, r=NR)
                    ps = ps_pool.tile([P, 512], f32, tag="ps")

                    if _is_sub2(bk):
                        # W[p, (r, el', bn)] = D[p, (gi, r, bn)] * M2[p, (el', bn)]
                        W = w_pool.tile([P, NR * 64], f16, tag="W2")
                        Wv = W[:].rearrange("p (r e b) -> p r e b", r=NR, e=8)
                        Db = Dv[:, gi, :, None, :].to_broadcast([P, NR, 8, 8])
                        Mb = M2v[:, None, :, :].to_broadcast([P, NR, 8, 8])
                        nc.vector.tensor_tensor(out=Wv, in0=Db, in1=Mb,
                                                op=mybir.AluOpType.mult)
                        for r in range(NR):
                            for h in range(2):
                                pr = slice(64 * h, 64 * h + 64)
                                nc.tensor.matmul(
                                    out=ps[pr, r * 18:(r + 1) * 18],
                                    lhsT=W[pr, r * 64:(r + 1) * 64],
                                    rhs=S[pr, (gi * NR + r) * 18:(gi * NR + r + 1) * 18],
                                    start=True, stop=True,
                                    tile_position=(64 * h, 64 * h),
                                )
                    else:
                        W = w_pool.tile([P, NR * 32], f16, tag="W")
                        Wv = W[:].rearrange("p (r e b) -> p r e b", r=NR, e=4)
                        Db = Dv[:, gi, :, None, :].to_broadcast([P, NR, 4, 8])
                        Mb = Mv[:, None, :, :].to_broadcast([P, NR, 4, 8])
                        nc.vector.tensor_tensor(out=Wv, in0=Db, in1=Mb,
                                                op=mybir.AluOpType.mult)
                        for r in range(NR):
                            for i in range(4):
                                pr = slice(32 * i, 32 * i + 32)
                                nc.tensor.matmul(
                                    out=ps[pr, r * 18:(r + 1) * 18],
                                    lhsT=W[pr, r * 32:(r + 1) * 32],
                                    rhs=S[pr, (gi * NR + r) * 18:(gi * NR + r + 1) * 18],
                                    start=True, stop=True,
                                    tile_position=(32 * i, 32 * i),
                                )

                    eslot = bk % EG
                    nc.scalar.copy(
                        out=ev[:, eslot * NR * 18:(eslot + 1) * NR * 18],
                        in_=ps[:, :NR * 18])

                    if eslot == EG - 1:
                        nc.sync.dma_start(
                            out=o_d[bk // EG].rearrange("p e r g -> p (e r g)"),
                            in_=ev[:])

    nc.compile()
    return nc


def _get_program():
    global _compiled
    if _compiled is None:
        _compiled = _build_program()
    return _compiled


def _make_masks():
    m = np.zeros((P, 96), np.float16)
    for p in range(P):
        el4 = (p % 32) // 8
        m[p, el4 * 8:(el4 + 1) * 8] = 1.0
        el8 = (p % 64) // 8
        m[p, 32 + el8 * 8:32 + (el8 + 1) * 8] = 1.0
    return m


_SUB2_BANKS = np.array([bk for bk in range(NBANK) if _is_sub2(bk)])
_SUB4_BANKS = np.array([bk for bk in range(NBANK) if not _is_sub2(bk)])


def _marshal_core(inputs_f16: np.ndarray, shpdx: np.ndarray,
                  elem_nodes: np.ndarray, c: int):
    """Build the d_in / s_in arrays for core c."""
    sl = slice(c * E_CORE, (c + 1) * E_CORE)
    en = elem_nodes[sl]                                   # [E, 8]
    disp = inputs_f16[:, en]                              # [B, E, 8, 2] f16
    # -> [E, k, (b, n)]
    dispc = np.ascontiguousarray(disp.transpose(1, 2, 0, 3)).reshape(E_CORE, 8, 8)
    dpad = np.zeros((E_PAD, 8, 8), np.float16)
    dpad[:E_CORE] = dispc
    spad = np.zeros((E_PAD, N_GP, 8, 2), np.float16)
    spad[:E_CORE] = shpdx[sl].astype(np.float16)

    # e = bk*EPB + r*16 + grp*per + el ; partition p = grp*per*8 + el*8 + k
    d_all = np.empty((NBANK, P, NR, 8), np.float16)
    s_all = np.empty((NBANK, P, NR, 18), np.float16)
    db = dpad.reshape(NBANK, NR, 16, 8, 8)                # bk r sub k bn
    sb = spad.reshape(NBANK, NR, 16, N_GP, 8, 2)          # bk r sub g k l
    for banks, ngrp, per in ((_SUB2_BANKS, 2, 8), (_SUB4_BANKS, 4, 4)):
        dv = db[banks].reshape(len(banks), NR, ngrp, per, 8, 8)
        dv = dv.transpose(0, 2, 3, 4, 1, 5)               # bk grp el k r bn
        d_all[banks] = dv.reshape(len(banks), P, NR, 8)
        sv = sb[banks].reshape(len(banks), NR, ngrp, per, N_GP, 8, 2)
        sv = sv.transpose(0, 2, 3, 5, 1, 4, 6)            # bk grp el k r g l
        s_all[banks] = sv.reshape(len(banks), P, NR, 18)

    d_in = np.ascontiguousarray(
        d_all.reshape(NBG, G, P, NR, 8).transpose(0, 2, 1, 3, 4))
    s_in = np.ascontiguousarray(
        s_all.reshape(NBG, G, P, NR, 18).transpose(0, 2, 1, 3, 4))
    return d_in, s_in


def _decode_core(o: np.ndarray) -> np.ndarray:
    """o: [NEVG, P, EG, NR, 18] fp16 -> strains [B, E_CORE*9, 3] f32."""
    # out partition p = grp*per*8 + el'*8 + (b*2+n)
    ob = o.reshape(NEVG, P, EG, NR, N_GP, 2).transpose(0, 2, 1, 3, 4, 5)
    ob = ob.reshape(NBANK, P, NR, N_GP, 2)                # bk p r g l
    dd = np.empty((B, E_PAD, N_GP, 2, 2), np.float16)
    ddv = dd.reshape(B, NBANK, NR, 16, N_GP, 2, 2)
    for banks, ngrp, per in ((_SUB2_BANKS, 2, 8), (_SUB4_BANKS, 4, 4)):
        ov = ob[banks].reshape(len(banks), ngrp, per, B, 2, NR, N_GP, 2)
        # -> b bk r (grp el) g n l
        ov = ov.transpose(3, 0, 5, 1, 2, 6, 4, 7)
        ddv[:, banks] = ov.reshape(B, len(banks), NR, 16, N_GP, 2, 2)
    dd = dd[:, :E_CORE].astype(np.float32)
    e_xx = dd[..., 0, 0]
    e_yy = dd[..., 1, 1]
    e_xy = 0.5 * (dd[..., 0, 1] + dd[..., 1, 0])
    e = np.stack([e_xx, e_yy, e_xy], axis=-1)             # [B, E, 9, 3]
    return e.reshape(B, E_CORE * N_GP, 3)


def kernel(inputs, shpdx, elem_nodes, _want_trace=False):
    nc = _get_program()

    inputs_f16 = inputs.astype(np.float16)
    mask = _make_masks()
    in_maps = []
    for c in range(N_CORES):
        d_in, s_in = _marshal_core(inputs_f16, shpdx, elem_nodes, c)
        in_maps.append({"d_in": d_in, "s_in": s_in, "mask": mask})

    core_ids = list(range(N_CORES))
    res = run_bass_kernel_spmd(nc, in_maps, core_ids, trace=_want_trace)

    outs = []
    for c in range(N_CORES):
        outs.append(_decode_core(np.asarray(res.results[c]["out"])))
    full = np.concatenate(outs, axis=1)                   # [B, N_ELEM*9, 3]
    if _want_trace:
        return full, res
    return full
